# revision 46
# baseline (speedup 1.0000x reference)
"""GAT (2-layer, 8-head) Trainium2 Bass kernel, SPMD over 8 NeuronCores. v2.

Strategy (dst-node-parallel, fp8 message tables, host-built one-hot mats):
  - Host: sort edges by dst; each core owns 1250 dst nodes (10 tiles of 128).
    Per tile, edges are padded to an even number of 128-slot chunks.
    Host precomputes:
      * xT      [512, 10000] bf16  pretransposed features (proj1 lhsT)
      * w1c/w2c [K, 1040]    bf16  fused [W_heads | a_sj | a_si]
      * src16   wrapped int16 gather indices (edge src, per core)
      * siI     wrapped int16 indices of the core's owned nodes (si rows)
      * oh8     [128, tot_chunks*128] fp8  one-hot scatter-matmul lhsT:
                oh[p, g*128+d] = (dstl[slot g*128+p] == d)
      * oh2b    [128, tot_chunks*128] bf16 per-chunk transpose of oh (maps
                dst-side scores si onto edge slots via small matmuls)
  - Device per core (single NEFF):
      proj: table[n] = [h fp8 1024B | sj bf16 16B | pad] (1280B rows),
            sit[n] = 256B row, si in first 16B. Replicated over all nodes.
      edges per owned tile: dma_gather 1280B rows by src; si gathered once
            per layer (owned rows, partition-aligned); logits -> exp;
            alpha-weighted scatter via fp8 DoubleRow matmuls (2 chunks per
            instruction, denominator rides the same weights);
            head-mean + ELU -> emb_loc.
      AllGather emb_locT (locally transposed) -> emb_allT [1024, 1250].
      layer 2 -> per-core partial graph-sum [1, 128].
  - Host: sum partials, LayerNorm + MLP head -> [16].
"""

import numpy as np
import ml_dtypes

BF16 = ml_dtypes.bfloat16
F8 = ml_dtypes.float8_e4m3

N_NODES = 10000
N_EDGES = 160000
N_FEAT = 512
HEADS = 8
HID = 128
OUT = 16

N_CORES = 8
P = 128

HD = HEADS * HID            # 1024
ROWB = 1280                 # table row bytes: 1024 fp8 h + 16 B bf16 sj + pad
SIROW = 256                 # si table row: 256 fp8 = 256 B (16 used: hi|lo)
PCOLS = HD + 16             # proj out cols [h 1024 | sj 8 | si 8]


# ----------------------------------------------------------------------------
# host-side prep
# ----------------------------------------------------------------------------

def _wrap_idx(idx_i16: np.ndarray) -> np.ndarray:
    """[n] int16 -> [128, n//16] wrapped layout for dma_gather."""
    n = idx_i16.shape[0]
    assert n % 16 == 0
    w = idx_i16.reshape(n // 16, 16).T
    return np.tile(w, (8, 1))


def host_prep(node_features, edge_src, edge_dst, W1, a1, W2, a2):
    nloc = N_NODES // N_CORES  # 1250
    T = -(-nloc // P)          # 10

    order = np.argsort(edge_dst, kind="stable")
    src_s = edge_src[order].astype(np.int64)
    dst_s = edge_dst[order].astype(np.int64)

    tile_cnt = np.zeros((N_CORES, T), dtype=np.int64)
    for c in range(N_CORES):
        base = c * nloc
        for t in range(T):
            n0 = base + t * P
            n1 = min(base + (t + 1) * P, (c + 1) * nloc)
            lo = np.searchsorted(dst_s, n0, side="left")
            hi = np.searchsorted(dst_s, n1, side="left")
            tile_cnt[c, t] = hi - lo
    C_t = []
    for t in range(T):
        c_ = int(-(-tile_cnt[:, t].max() // P))
        C_t.append(c_ + (c_ % 2))  # even, for DoubleRow chunk pairs
    assert max(C_t) * 8 <= 176, f"C_t too large: {C_t}"

    SLOTS = [c * P for c in C_t]
    tot_chunks = sum(C_t)
    tot_slots = tot_chunks * P

    xT = np.ascontiguousarray(node_features.astype(np.float32).T.astype(BF16))
    xT8 = np.ascontiguousarray(node_features.astype(np.float32).T.astype(F8))

    def wcat(W, a, K):
        w = np.transpose(W, (2, 0, 1)).reshape(K, HEADS * HID)
        si = np.einsum("hdf,hd->fh", W, a[:, :HID])   # dst-side score
        sj = np.einsum("hdf,hd->fh", W, a[:, HID:])   # src-side score
        return np.ascontiguousarray(
            np.concatenate([w, sj, si], axis=1).astype(BF16))

    w1c = wcat(W1, a1, N_FEAT)
    w1c8 = np.ascontiguousarray(w1c.astype(np.float32).astype(F8))
    w2c = wcat(W2, a2, HID)

    in_maps = []
    for c in range(N_CORES):
        base = c * nloc
        src_pad = np.zeros(tot_slots, dtype=np.int64)
        dstl_pad = np.full(tot_slots, 30000, dtype=np.int64)
        off = 0
        for t in range(T):
            n0 = base + t * P
            n1 = min(base + (t + 1) * P, (c + 1) * nloc)
            lo = np.searchsorted(dst_s, n0, side="left")
            hi = np.searchsorted(dst_s, n1, side="left")
            k = hi - lo
            sub = np.argsort(src_s[lo:hi], kind="stable")  # gather locality
            src_pad[off:off + k] = src_s[lo:hi][sub]
            dstl_pad[off:off + k] = (dst_s[lo:hi][sub] - n0)
            off += SLOTS[t]
        assert off == tot_slots

        src16 = np.concatenate(
            [_wrap_idx(src_pad[sum(SLOTS[:t]):sum(SLOTS[:t + 1])].astype(np.int16))
             for t in range(T)], axis=1)

        # owned-node si indices: slot t*128+d -> node base+t*128+d (pad -> 0;
        # pads must be VALID indices so gathered rows stay finite)
        own = np.where(np.arange(T * P) < nloc, base + np.arange(T * P), 0)
        siI = _wrap_idx(own.astype(np.int16))

        # one-hot matrices: slot s = chunk g * 128 + partition p
        dstl_pg = dstl_pad.reshape(tot_chunks, P)            # [g, p]
        dgrid = np.arange(P)
        oh = (dstl_pg[:, :, None] == dgrid[None, None, :])   # [g, p, d]
        oh8 = np.ascontiguousarray(
            oh.transpose(1, 0, 2).reshape(P, tot_chunks * P).astype(F8))
        oh2b = np.ascontiguousarray(
            oh.transpose(2, 0, 1).reshape(P, tot_chunks * P).astype(BF16))

        in_maps.append({
            "xT": xT,
            "xT8": xT8,
            "w1c8": w1c8,
            "w1c": w1c,
            "w2c": w2c,
            "src16": np.ascontiguousarray(src16),
            "siI": np.ascontiguousarray(siI),
            "oh8": oh8,
            "oh2b": oh2b,
        })

    meta = {"T": T, "C_t": C_t, "nloc": nloc, "tot_chunks": tot_chunks}
    return in_maps, meta


# ----------------------------------------------------------------------------
# device program
# ----------------------------------------------------------------------------

def build_program(meta, debug=False, stages=5, iters=1, ablate="",
                  proj_fp8=False):
    import concourse.bacc as bacc
    import concourse.mybir as mybir
    import concourse.tile as tile
    from concourse.library_config import mlp

    dt = mybir.dt
    Alu = mybir.AluOpType
    Act = mybir.ActivationFunctionType
    DR = mybir.MatmulPerfMode.DoubleRow

    T = meta["T"]
    C_t = meta["C_t"]
    nloc = meta["nloc"]
    tot_chunks = meta["tot_chunks"]

    K1 = N_FEAT // P  # 4
    RB = 512

    nc = bacc.Bacc("TRN2", num_devices=N_CORES, num_swdge_queues=2,
                   dynamic_dma_scratch_size=49152)

    xT = nc.dram_tensor("xT", [N_FEAT, N_NODES], dt.bfloat16,
                        kind="ExternalInput")
    xT8 = nc.dram_tensor("xT8", [N_FEAT, N_NODES], dt.float8e4,
                         kind="ExternalInput")
    w1c8 = nc.dram_tensor("w1c8", [N_FEAT, PCOLS], dt.float8e4,
                          kind="ExternalInput")
    w1c = nc.dram_tensor("w1c", [N_FEAT, PCOLS], dt.bfloat16,
                         kind="ExternalInput")
    w2c = nc.dram_tensor("w2c", [HID, PCOLS], dt.bfloat16,
                         kind="ExternalInput")
    src16 = nc.dram_tensor("src16", [P, tot_chunks * 8], dt.int16,
                           kind="ExternalInput")
    siI16 = nc.dram_tensor("siI", [P, T * 8], dt.int16, kind="ExternalInput")
    oh8 = nc.dram_tensor("oh8", [P, tot_chunks * P], dt.float8e4,
                         kind="ExternalInput")
    oh2b = nc.dram_tensor("oh2b", [P, tot_chunks * P], dt.bfloat16,
                          kind="ExternalInput")

    out_vec = nc.dram_tensor("out_vec", [1, HID], dt.float32,
                             kind="ExternalOutput")
    dbg = {}
    if debug:
        dbg["embloc"] = nc.dram_tensor("dbg_embloc", [nloc, HID], dt.float32,
                                       kind="ExternalOutput")
        dbg["tab"] = nc.dram_tensor("dbg_tab", [P, ROWB], dt.float32,
                                    kind="ExternalOutput")
        dbg["g"] = nc.dram_tensor("dbg_g", [P, HD], dt.float32,
                                  kind="ExternalOutput")
        dbg["sj"] = nc.dram_tensor("dbg_sj", [P, 8], dt.float32,
                                   kind="ExternalOutput")
        dbg["lg"] = nc.dram_tensor("dbg_lg", [P, 144], dt.float32,
                                   kind="ExternalOutput")
        dbg["ex"] = nc.dram_tensor("dbg_ex", [P, 144], dt.float32,
                                   kind="ExternalOutput")
        dbg["S"] = nc.dram_tensor("dbg_S", [P, HID], dt.float32,
                                  kind="ExternalOutput")
        dbg["sig"] = nc.dram_tensor("dbg_sig", [P, 16], dt.float32,
                                    kind="ExternalOutput")

    table1 = nc.dram_tensor("table1", [N_NODES, ROWB], dt.float8e4)
    sit1 = nc.dram_tensor("sit1", [N_NODES, SIROW], dt.float8e4)
    table2 = nc.dram_tensor("table2", [N_NODES, ROWB], dt.float8e4)
    sit2 = nc.dram_tensor("sit2", [N_NODES, SIROW], dt.float8e4)
    HN = 640  # AG half-split: tiles 0-4 -> A (owned nodes 0:640), 5-9 -> B
    emb_locA = nc.dram_tensor("emb_locA", [HN, HID], dt.bfloat16)
    emb_locB = nc.dram_tensor("emb_locB", [HN, HID], dt.bfloat16)
    emb_locTA = nc.dram_tensor("emb_locTA", [P, HN], dt.bfloat16)
    emb_locTB = nc.dram_tensor("emb_locTB", [P, HN], dt.bfloat16)
    emb_allTA = nc.dram_tensor("emb_allTA", [N_CORES * P, HN], dt.bfloat16,
                               addr_space="Shared")
    emb_allTB = nc.dram_tensor("emb_allTB", [N_CORES * P, HN], dt.bfloat16,
                               addr_space="Shared")

    with tile.TileContext(nc) as tc:
        with (
            tc.tile_pool(name="const", bufs=1) as cpool,
            tc.tile_pool(name="xtp", bufs=5) as tpool,
            tc.tile_pool(name="work", bufs=2) as wpool,
            tc.tile_pool(name="chunk", bufs=3) as kpool,
            tc.tile_pool(name="post", bufs=2) as opool,
            tc.tile_pool(name="psum", bufs=2, space="PSUM") as pspool,
            tc.tile_pool(name="psg", bufs=1, space="PSUM") as psg,
        ):
            nc.gpsimd.load_library(mlp)

            # ---- constants (loaded once per NEFF) ----
            ones_col = cpool.tile([P, 1], dt.bfloat16)
            nc.gpsimd.memset(ones_col[:], 1.0)
            nlog16 = cpool.tile([P, 1], dt.float32)
            nc.gpsimd.memset(nlog16[:], -2.772588722239781)

            if not proj_fp8:
                w1s = cpool.tile([P, K1 * PCOLS], dt.bfloat16)
                nc.sync.dma_start(
                    out=w1s[:].rearrange("p (k c) -> p k c", k=K1),
                    in_=w1c[:].rearrange("(k p) c -> p k c", p=P))
            else:
                w1s = None
            w2s = cpool.tile([P, PCOLS], dt.bfloat16)
            nc.sync.dma_start(out=w2s[:], in_=w2c[:])
            if proj_fp8:
                w1s8 = cpool.tile([P, K1 * PCOLS], dt.float8e4)
                nc.sync.dma_start(
                    out=w1s8[:].rearrange("p (k c) -> p k c", k=K1),
                    in_=w1c8[:].rearrange("(k p) c -> p k c", p=P))

            srcI = cpool.tile([P, tot_chunks * 8], dt.int16)
            nc.sync.dma_start(out=srcI[:], in_=src16[:])
            siIS = cpool.tile([P, T * 8], dt.int16)
            nc.sync.dma_start(out=siIS[:], in_=siI16[:])
            ohS = cpool.tile([P, tot_chunks * P], dt.float8e4)
            nc.scalar.dma_start(out=ohS[:], in_=oh8[:])
            oh2S = cpool.tile([P, tot_chunks * P], dt.bfloat16)
            nc.scalar.dma_start(out=oh2S[:], in_=oh2b[:])

            # ---------------- projection ----------------
            # blocks: list of (r0 = abs node base, rn, src_tensor, prow, pcol)
            # src: lhsT slab src_tensor[prow:prow+128, pcol:pcol+rn]; for
            # layer 1 (K=4 chunks) src_tensor is None -> xT k-chunks.
            def project(layer, table, sit, blocks):
                K = K1 if layer == 1 else 1
                wtile = w1s if layer == 1 else w2s
                SBMAX = 5
                fp8_1 = proj_fp8 and layer == 1
                for (r0, rn, srcT, prow, pcol) in blocks:
                    xTs = []
                    if not fp8_1:
                        for k in range(K):
                            t_ = tpool.tile([P, SBMAX * P], dt.bfloat16,
                                            tag="xT")
                            if layer == 1:
                                eng = nc.sync if k % 2 == 0 else nc.scalar
                                eng.dma_start(
                                    out=t_[:, :rn],
                                    in_=xT[k * P:(k + 1) * P, r0:r0 + rn])
                            else:
                                nc.sync.dma_start(
                                    out=t_[:, :rn],
                                    in_=srcT[prow:prow + P, pcol:pcol + rn])
                            xTs.append(t_)
                    if fp8_1:
                        x8s = []
                        for kp in range(K1 // 2):
                            t8 = tpool.tile([P, 2, SBMAX * P], dt.float8e4,
                                            tag="xT8")
                            for j in (0, 1):
                                k = kp * 2 + j
                                nc.scalar.dma_start(
                                    out=t8[:, j, :rn],
                                    in_=xT8[k * P:(k + 1) * P, r0:r0 + rn])
                            x8s.append(t8)
                    SB = -(-rn // P)
                    row = wpool.tile([P, SBMAX, ROWB], dt.float8e4, tag="row")
                    sirow = wpool.tile([P, SBMAX, 16], dt.float8e4, tag="sirow")
                    for s in range(SB):
                        if s * P >= rn:
                            break
                        nn = min(P, rn - s * P)
                        ps = pspool.tile([P, HD], dt.float32, tag="ps")
                        sd = pspool.tile([P, 192], dt.float32, tag="sd")
                        if fp8_1:
                            w8r = w1s8[:].rearrange("p (k c) -> p k c", k=K1)
                            for kp in range(K1 // 2):
                                lhsT = x8s[kp][:, :, s * P:s * P + nn]
                                rhs = w8r[:, kp * 2:kp * 2 + 2, :]
                                st, sp = (kp == 0), (kp == K1 // 2 - 1)
                                nc.tensor.matmul(ps[:nn, 0:512], lhsT=lhsT,
                                                 rhs=rhs[:, :, 0:512],
                                                 start=st, stop=sp, perf_mode=DR)
                                nc.tensor.matmul(ps[:nn, 512:1024], lhsT=lhsT,
                                                 rhs=rhs[:, :, 512:1024],
                                                 start=st, stop=sp, perf_mode=DR)
                                nc.tensor.matmul(sd[:nn, 0:16], lhsT=lhsT,
                                                 rhs=rhs[:, :, 1024:1040],
                                                 start=st, stop=sp, perf_mode=DR)
                        else:
                            for k in range(K):
                                lhsT = xTs[k][:, s * P:s * P + nn]
                                rhs = wtile[:, k * PCOLS:(k + 1) * PCOLS] \
                                    if K > 1 else wtile[:]
                                st, sp = (k == 0), (k == K - 1)
                                nc.tensor.matmul(ps[:nn, 0:512], lhsT=lhsT,
                                                 rhs=rhs[:, 0:512], start=st,
                                                 stop=sp)
                                nc.tensor.matmul(ps[:nn, 512:1024], lhsT=lhsT,
                                                 rhs=rhs[:, 512:1024], start=st,
                                                 stop=sp)
                                nc.tensor.matmul(sd[:nn, 0:16], lhsT=lhsT,
                                                 rhs=rhs[:, 1024:1040], start=st,
                                                 stop=sp)
                        nc.vector.tensor_copy(row[:nn, s, 0:768], ps[:nn, 0:768])
                        nc.scalar.activation(row[:nn, s, 768:1024],
                                             ps[:nn, 768:1024], Act.Copy)
                        nc.vector.tensor_copy(
                            row[:nn, s, 1024:1040].bitcast(dt.bfloat16),
                            sd[:nn, 0:8])
                        # si stored bf16 (bitcast into the fp8 row bytes)
                        nc.vector.tensor_copy(
                            sirow[:nn, s, 0:16].bitcast(dt.bfloat16),
                            sd[:nn, 8:16])
                    if rn % P == 0:
                        nc.scalar.dma_start(
                            out=table[r0:r0 + rn, :].rearrange(
                                "(a p) c -> p a c", p=P),
                            in_=row[:, 0:SB, :])
                        nc.sync.dma_start(
                            out=sit[r0:r0 + rn, 0:16].rearrange(
                                "(a p) c -> p a c", p=P),
                            in_=sirow[:, 0:SB, 0:16])
                    else:
                        for s in range(SB):
                            if s * P >= rn:
                                break
                            nn = min(P, rn - s * P)
                            n0 = r0 + s * P
                            nc.scalar.dma_start(out=table[n0:n0 + nn, :],
                                                in_=row[:nn, s, :])
                            nc.sync.dma_start(out=sit[n0:n0 + nn, 0:16],
                                              in_=sirow[:nn, s, 0:16])

            # ---------------- edge phase ----------------
            def edges(table, sit, layer):
                gps = psg.tile([1, HID], dt.float32, tag="gsum", name="gps") \
                    if layer == 2 else None
                ebuf = cpool.tile([P, T * HID], dt.bfloat16, name="ebuf") \
                    if layer == 2 else None

                # owned-node si rows, partition-aligned: siG[d, t, :] is the
                # si row of node base + t*128 + d (bytes 0:16 = 8 x bf16 si)
                siG = wpool.tile([P, T, SIROW], dt.float8e4, tag="siG")
                nc.gpsimd.dma_gather(
                    siG[:], sit[:], siIS[:], T * P, T * P, SIROW,
                    single_packet=False, queue_num=0)
                if debug and layer == 1:
                    dsg = opool.tile([P, 16], dt.float32, tag="dsg", name="dsg")
                    nc.vector.tensor_copy(dsg[:], siG[:, 0, 0:16])
                    nc.sync.dma_start(out=dbg["sig"][:], in_=dsg[:])

                for t in range(T):
                    C = C_t[t]
                    ioff = sum(C_t[:t])
                    n0t = t * P
                    nn_t = min(P, nloc - n0t)
                    G = wpool.tile([P, C, ROWB], dt.float8e4, tag="G")
                    if ablate == "nogather":
                        nc.sync.dma_start(
                            out=G[:],
                            in_=table[0:C * P, :].rearrange(
                                "(c p) b -> p c b", p=P))
                    else:
                        Ch = C // 2
                        nc.gpsimd.dma_gather(
                            G[:, 0:Ch, :], table[:],
                            srcI[:, ioff * 8:(ioff + Ch) * 8],
                            Ch * P, Ch * P, ROWB, single_packet=False,
                            queue_num=0)
                        nc.gpsimd.dma_gather(
                            G[:, Ch:C, :], table[:],
                            srcI[:, (ioff + Ch) * 8:(ioff + C) * 8],
                            (C - Ch) * P, (C - Ch) * P, ROWB,
                            single_packet=False, queue_num=1)

                    sd = pspool.tile([P, 192], dt.float32, tag="sd")
                    # distribute dst-side si onto edge slots: per chunk,
                    # sd[slot, c*8:(c+1)*8] = oh2_c.T @ (si_hi + si_lo)
                    if ablate != "nosi":
                        for c in range(C):
                            nc.tensor.matmul(
                                sd[:, c * 8:(c + 1) * 8],
                                lhsT=oh2S[:, (ioff + c) * P:(ioff + c + 1) * P],
                                rhs=siG[:, t, 0:16].bitcast(dt.bfloat16),
                                start=True, stop=True)
                    # logits = si + sj; exp(leaky_relu(.)) — logits are O(5),
                    # no max-subtraction needed
                    LG = wpool.tile([P, C, 8], dt.float32, tag="LG")
                    if ablate == "nosi":
                        nc.vector.tensor_copy(
                            LG[:], G[:, :, 1024:1040].bitcast(dt.bfloat16))
                    else:
                        nc.vector.tensor_tensor(
                            out=LG[:],
                            in0=sd[:, 0:C * 8].rearrange("p (c e) -> p c e", c=C),
                            in1=G[:, :, 1024:1040].bitcast(dt.bfloat16),
                            op=Alu.add)
                    MX = wpool.tile([P, C * 8], dt.float32, tag="MX")
                    nc.scalar.activation(MX[:],
                                         LG[:].rearrange("p c e -> p (c e)"),
                                         Act.Lrelu, alpha=0.01)
                    # exp scaled by 1/16 so msg = EX*G stays under the fp8
                    # e4m3 max (240); alpha normalization cancels the scale
                    EX = wpool.tile([P, C * 8], dt.float32, tag="EX")
                    nc.scalar.activation(EX[:], MX[:], Act.Exp,
                                         bias=nlog16[:])
                    EXf8 = wpool.tile([P, C, 8], dt.float8e4, tag="EXf8")
                    nc.vector.tensor_copy(
                        EXf8[:], EX[:].rearrange("p (c e) -> p c e", c=C))
                    if debug and layer == 1 and t == 0:
                        dg = wpool.tile([P, HD], dt.float32, tag="dg", name="dg")
                        nc.vector.tensor_copy(dg[:], G[:, 0, 0:HD])
                        nc.sync.dma_start(out=dbg["g"][:], in_=dg[:])
                        dsj = opool.tile([P, 8], dt.float32, tag="dsj",
                                         name="dsj")
                        nc.vector.tensor_copy(
                            dsj[:], G[:, 0, 1024:1040].bitcast(dt.bfloat16))
                        nc.sync.dma_start(out=dbg["sj"][:], in_=dsj[:])
                        dlg = wpool.tile([P, 144], dt.float32, tag="dlg",
                                         name="dlg")
                        nc.vector.tensor_copy(
                            dlg[:], LG[:].rearrange("p c e -> p (c e)")[:, 0:144])
                        nc.sync.dma_start(out=dbg["lg"][:], in_=dlg[:])
                        dex = wpool.tile([P, 144], dt.float32, tag="dex",
                                         name="dex")
                        nc.vector.tensor_copy(dex[:], EX[:, 0:144])
                        nc.sync.dma_start(out=dbg["ex"][:], in_=dex[:])

                    ps = pspool.tile([P, HD], dt.float32, tag="ps")
                    for cp in range(C // 2):
                        c0 = 2 * cp
                        msg = kpool.tile([P, 2, HD], dt.float8e4, tag="msg")
                        if ablate != "nomsg" or cp == 0:
                            for j in (0, 1):
                                c = c0 + j
                                exb = EX[:, c * 8:(c + 1) * 8] \
                                    .broadcast_to([P, 8, HID])
                                nc.vector.tensor_tensor(
                                    out=msg[:, j, :].rearrange(
                                        "p (h f) -> p h f", h=HEADS),
                                    in0=G[:, c, 0:HD].rearrange(
                                        "p (h f) -> p h f", h=HEADS),
                                    in1=exb, op=Alu.mult)
                        ohp = ohS[:, (ioff + c0) * P:(ioff + c0 + 2) * P] \
                            .rearrange("p (two d) -> p two d", two=2)
                        st, sp = (cp == 0), (cp == C // 2 - 1)
                        if ablate in ("nomm", "nomsg"):
                            if cp == 0:
                                nc.tensor.matmul(ps[:, 0:512], lhsT=ohp,
                                                 rhs=msg[:, :, 0:512],
                                                 start=True, stop=True,
                                                 perf_mode=DR)
                                nc.tensor.matmul(ps[:, 512:1024], lhsT=ohp,
                                                 rhs=msg[:, :, 512:1024],
                                                 start=True, stop=True,
                                                 perf_mode=DR)
                        else:
                            nc.tensor.matmul(ps[:, 0:512], lhsT=ohp,
                                             rhs=msg[:, :, 0:512], start=st,
                                             stop=sp, perf_mode=DR)
                            nc.tensor.matmul(ps[:, 512:1024], lhsT=ohp,
                                             rhs=msg[:, :, 512:1024], start=st,
                                             stop=sp, perf_mode=DR)
                        nc.tensor.matmul(sd[:, 176:184], lhsT=ohp,
                                         rhs=EXf8[:, c0:c0 + 2, :], start=st,
                                         stop=sp, perf_mode=DR)

                    # ---- postprocess tile ----
                    den = opool.tile([P, 8], dt.float32, tag="den")
                    nc.vector.tensor_scalar(out=den[:], in0=sd[:, 176:184],
                                            scalar1=float(HEADS), scalar2=1e-30,
                                            op0=Alu.mult, op1=Alu.max)
                    rec = opool.tile([P, 8], dt.float32, tag="rec")
                    nc.vector.reciprocal(rec[:], den[:])
                    Sa = opool.tile([P, HID], dt.float32, tag="Sa")
                    Sb = opool.tile([P, HID], dt.float32, tag="Sb")
                    nc.vector.tensor_scalar_mul(Sa[:], ps[:, 0:HID], rec[:, 0:1])
                    for h in range(1, HEADS):
                        tmp = opool.tile([P, HID], dt.float32, tag="tmp")
                        nc.vector.tensor_scalar_mul(
                            tmp[:], ps[:, h * HID:(h + 1) * HID], rec[:, h:h + 1])
                        a, b = (Sa, Sb) if h % 2 == 1 else (Sb, Sa)
                        nc.vector.tensor_tensor(out=b[:], in0=a[:], in1=tmp[:],
                                                op=Alu.add)
                    S = Sb if HEADS % 2 == 0 else Sa
                    if debug and layer == 1 and t == 0:
                        dS = wpool.tile([P, HID], dt.float32, tag="dS",
                                        name="dS")
                        nc.vector.tensor_copy(dS[:], S[:])
                        nc.sync.dma_start(out=dbg["S"][:], in_=dS[:])
                    # elu(S) = exp(min(S,0)) - 1 + max(S,0)
                    neg = opool.tile([P, HID], dt.float32, tag="neg")
                    nc.vector.tensor_scalar_min(neg[:], S[:], 0.0)
                    en = opool.tile([P, HID], dt.float32, tag="en")
                    nc.scalar.activation(en[:], neg[:], Act.Exp)
                    pos = opool.tile([P, HID], dt.float32, tag="pos")
                    nc.vector.tensor_scalar_max(pos[:], S[:], 0.0)
                    eadd = opool.tile([P, HID], dt.float32, tag="eadd")
                    nc.vector.tensor_tensor(out=eadd[:], in0=en[:], in1=pos[:],
                                            op=Alu.add)
                    if layer == 1:
                        ebf = opool.tile([P, HID], dt.bfloat16, tag="ebf")
                        nc.vector.tensor_scalar_add(ebf[:], eadd[:], -1.0)
                        edst = emb_locA if t < 5 else emb_locB
                        e0 = n0t if t < 5 else n0t - HN
                        nc.sync.dma_start(out=edst[e0:e0 + nn_t, :],
                                          in_=ebf[:nn_t, :])
                        # transpose + AllGather each half as soon as its five
                        # tiles are done; AG-A overlaps edges1 tiles 5-9,
                        # AG-B overlaps proj2's A-half
                        if stages >= 3 and t in (4, T - 1):
                            eloc, elocT, eallT = (
                                (emb_locA, emb_locTA, emb_allTA) if t == 4
                                else (emb_locB, emb_locTB, emb_allTB))
                            trh = wpool.tile([P, HN], dt.bfloat16, tag="trh")
                            nc.sync.dma_start_transpose(trh[:, 0:RB],
                                                        eloc[0:RB, :])
                            nc.sync.dma_start_transpose(trh[:, RB:HN],
                                                        eloc[RB:HN, :])
                            nc.scalar.dma_start(out=elocT[:], in_=trh[:])
                            nc.gpsimd.collective_compute(
                                "AllGather", Alu.bypass,
                                ins=[elocT[:]], outs=[eallT[:]],
                                replica_groups=[list(range(N_CORES))])
                    else:
                        nc.vector.tensor_scalar_add(
                            ebuf[:, t * HID:(t + 1) * HID], eadd[:], -1.0)
                if layer == 2:
                    for t in range(T):
                        nn_t = min(P, nloc - t * P)
                        nc.tensor.matmul(gps[0:1, :], lhsT=ones_col[:nn_t, :],
                                         rhs=ebuf[:nn_t, t * HID:(t + 1) * HID],
                                         start=(t == 0), stop=(t == T - 1))
                return gps

            # ---------------- main flow ----------------
            def zero_out_vec():
                z = opool.tile([1, HID], dt.float32, tag="gout", name="z")
                nc.gpsimd.memset(z[:], 0.0)
                nc.sync.dma_start(out=out_vec[:], in_=z[:])

            def flow():
                blocks1 = [(b * RB, min(RB, N_NODES - b * RB), None, 0, 0)
                           for b in range(-(-N_NODES // RB))]
                project(1, table1, sit1, blocks1)
                if debug:
                    tf8 = wpool.tile([P, ROWB], dt.float8e4, tag="tf8",
                                     name="tf8")
                    nc.sync.dma_start(out=tf8[:], in_=table1[0:P, :])
                    t32 = wpool.tile([P, ROWB], dt.float32, tag="t32",
                                     name="t32")
                    nc.vector.tensor_copy(t32[:], tf8[:])
                    nc.sync.dma_start(out=dbg["tab"][:], in_=t32[:])
                if stages >= 2:
                    edges(table1, sit1, layer=1)
                    if debug:
                        for b in range(-(-nloc // P)):
                            n0 = b * P
                            nn = min(P, nloc - n0)
                            src = emb_locA if n0 < HN else emb_locB
                            o0 = n0 if n0 < HN else n0 - HN
                            te = wpool.tile([P, HID], dt.float32, tag="dbgt")
                            tb = wpool.tile([P, HID], dt.bfloat16, tag="dbgb")
                            nc.sync.dma_start(out=tb[:nn, :],
                                              in_=src[o0:o0 + nn, :])
                            nc.vector.tensor_copy(te[:nn, :], tb[:nn, :])
                            nc.sync.dma_start(out=dbg["embloc"][n0:n0 + nn, :],
                                              in_=te[:nn, :])
                if stages >= 4:
                    blocks2 = []
                    for half, srcT, hn in ((0, emb_allTA, HN),
                                           (1, emb_allTB, nloc - HN)):
                        for cblk in range(N_CORES):
                            blocks2.append((cblk * nloc + half * HN, hn,
                                            srcT, cblk * P, 0))
                    project(2, table2, sit2, blocks2)
                if stages >= 5:
                    gps = edges(table2, sit2, layer=2)
                    gout = opool.tile([1, HID], dt.float32, tag="gout")
                    nc.vector.tensor_copy(gout[:], gps[:])
                    nc.sync.dma_start(out=out_vec[:], in_=gout[:])
                else:
                    zero_out_vec()

            for _it in range(iters):
                flow()

    nc.compile()
    return nc


# ----------------------------------------------------------------------------
# top-level kernel
# ----------------------------------------------------------------------------

_CACHE = {}


def _run_device(in_maps, meta):
    from concourse.bass_utils import run_bass_kernel_spmd
    key = "prog"
    if key not in _CACHE:
        _CACHE[key] = build_program(meta)
    nc = _CACHE[key]
    res = run_bass_kernel_spmd(nc, in_maps, core_ids=list(range(N_CORES)))
    return res


def host_finish(partials, ln_g, ln_b, Wl1, bl1, Wl2, bl2, Wl3, bl3):
    g = partials.sum(axis=0) / np.float64(N_NODES)
    mu = g.mean()
    var = ((g - mu) ** 2).mean()
    gn = (g - mu) / np.sqrt(var + 1e-5) * ln_g + ln_b
    x = Wl1 @ gn + bl1
    x = np.maximum(x, 0.01 * x)
    x = Wl2 @ x + bl2
    x = np.maximum(x, 0.01 * x)
    x = Wl3 @ x + bl3
    return np.maximum(x, 0.0).astype(np.float32)


def kernel(node_features, edge_src, edge_dst, W1, a1, W2, a2,
           ln_g, ln_b, Wl1, bl1, Wl2, bl2, Wl3, bl3):
    node_features = np.asarray(node_features, dtype=np.float32)
    edge_src = np.asarray(edge_src, dtype=np.int32)
    edge_dst = np.asarray(edge_dst, dtype=np.int32)
    in_maps, meta = host_prep(node_features, edge_src, edge_dst,
                              np.asarray(W1, np.float32), np.asarray(a1, np.float32),
                              np.asarray(W2, np.float32), np.asarray(a2, np.float32))
    res = _run_device(in_maps, meta)
    partials = np.stack([res.results[c]["out_vec"][0] for c in range(N_CORES)])
    return host_finish(partials.astype(np.float64),
                       np.asarray(ln_g, np.float64), np.asarray(ln_b, np.float64),
                       np.asarray(Wl1, np.float64), np.asarray(bl1, np.float64),
                       np.asarray(Wl2, np.float64), np.asarray(bl2, np.float64),
                       np.asarray(Wl3, np.float64), np.asarray(bl3, np.float64))
